# revision 17
# baseline (speedup 1.0000x reference)
"""Masked causal self-attention (single head) on 8 Trainium2 NeuronCores.

Problem: x[4,4096,1024], mask[4,4096] (key padding), Wq/Wk/Wv[128,1024],
bq/bk/bv[128] -> out[4,4096,128]:
    q = x@Wq.T+bq; k = x@Wk.T+bk; v = x@Wv.T+bv
    out = softmax(causal_mask(q@k.T/sqrt(128)) + key_padding) @ v

Sharding (SPMD, one program on 8 cores): core c = (batch b=c//2, parity
p=c%2). Each core computes K/V for its full batch (replicated within the
pair) and handles the interleaved query 128-row tiles {2*t+p : t in 0..15}
— interleaving balances the causal (triangular) work between the pair.

Device algorithm per core:
  - All matmuls run in float32r (single-pass fp32, 1 cycle/row at moving
    free-dim >= 256, vs 4 cycles/row for exact fp32) via AP bitcasts.
  - Projections are computed transposed ([head, seq] layout) with the
    d-contraction on partitions: K^T/V^T/Q^T = W.T-chunks @ x^T-chunks,
    accumulated in PSUM over 8 d-chunks. Biases are folded in during the
    PSUM->SBUF eviction (per-partition scalar add). The 1/sqrt(128) score
    scale is folded into Wq on the host.
  - V^T is transposed back to [seq,head] tiles with the PE (stationary
    operand of attn@V).
  - Scores are computed transposed: S^T[k,q] = (K^T-tile).T @ Q^T-chunk.
    exp() runs on the scalar engine straight out of PSUM; its per-partition
    bias argument carries the key-padding mask (-1e30 for masked keys).
    Softmax max-subtraction is skipped: scores are ~N(0,1) by construction
    (matches jax softmax mathematically; no overflow in fp32).
  - The causal mask is data-driven so the program is core-uniform: a 0/1
    tile M = (qg >= kg) (qg per-core query indices from DRAM, kg an iota)
    multiplies exp(S^T) for the ~diagonal k-tiles only (into a fresh tile,
    pt2, so every tile has a single writer engine).
  - attn@V accumulates transposed, a whole 512-query chunk at a time:
    outT[h, qchunk] += V_kt.T @ PT_kt and den[*, qchunk] += ones.T @ PT_kt
    (the ones-matmul gives the softmax denominator replicated across
    partitions, so normalization is a plain elementwise multiply).
    The output leaves the device as [H, NQ]; the host transposes.

Hardware instructions carry a single semaphore-wait slot; Bacc.compile()
legalizes multi-wait instructions (generate_event_semaphores).
"""

import sys

sys.path.insert(0, "/opt/trn_rl_repo")

import numpy as np

import concourse.bass as bass
import concourse.bacc as bacc
import concourse.tile as tile
from concourse import mybir
from concourse.masks import make_identity
from concourse import bass_utils

F32 = mybir.dt.float32
F32R = mybir.dt.float32r
B, S, D, H = 4, 4096, 1024, 128
NQ = S // 2          # queries owned per core (2048)
DC = D // 128        # 8 d-chunks
SCH = S // 512       # 8 seq chunks of 512
NKT = S // 128       # 32 key tiles
NEG = -1.0e30


def _build_program():
    nc = bacc.Bacc("TRN2", target_bir_lowering=False)

    xT_d = nc.dram_tensor("xT", [D, S], F32R, kind="ExternalInput")
    xqT_d = nc.dram_tensor("xqT", [D, NQ], F32R, kind="ExternalInput")
    wqT_d = nc.dram_tensor("wqT", [128, DC * H], F32R, kind="ExternalInput")
    wkT_d = nc.dram_tensor("wkT", [128, DC * H], F32R, kind="ExternalInput")
    wvT_d = nc.dram_tensor("wvT", [128, DC * H], F32R, kind="ExternalInput")
    bq_d = nc.dram_tensor("bq", [H, 1], F32, kind="ExternalInput")
    bk_d = nc.dram_tensor("bk", [H, 1], F32, kind="ExternalInput")
    bv_d = nc.dram_tensor("bv", [H, 1], F32, kind="ExternalInput")
    mb_d = nc.dram_tensor("maskbias", [128, NKT], F32, kind="ExternalInput")
    qg_d = nc.dram_tensor("qg", [4, 512], F32, kind="ExternalInput")
    o_d = nc.dram_tensor("o", [H, NQ], F32, kind="ExternalOutput")

    with tile.TileContext(nc) as tc:
        with (
            tc.tile_pool(name="consts", bufs=1) as consts,
            tc.tile_pool(name="big", bufs=1) as big,
            tc.tile_pool(name="vtiles", bufs=NKT) as vtiles,
            tc.tile_pool(name="ptp", bufs=6) as ptp,
            tc.tile_pool(name="pt2p", bufs=4) as pt2p,
        ):
            # ---- constants ----
            ident = consts.tile([128, 128], F32)
            make_identity(nc, ident)
            ones_f = consts.tile([128, 128], F32, tag="ones_f")
            nc.vector.memset(ones_f, 1.0)
            ones = consts.tile([128, 128], F32R)
            nc.vector.tensor_copy(ones, ones_f)
            kg = consts.tile([128, NKT], F32)
            nc.gpsimd.iota(
                kg, pattern=[[128, NKT]], base=0, channel_multiplier=1,
                allow_small_or_imprecise_dtypes=True,
            )
            mb = consts.tile([128, NKT], F32)
            nc.sync.dma_start(out=mb, in_=mb_d[:, :])
            qg_b = []
            for j in range(4):
                t = consts.tile([128, 512], F32, tag="qg_b")
                row = qg_d[j, :]
                nc.gpsimd.dma_start(
                    out=t,
                    in_=bass.AP(tensor=row.tensor, offset=row.offset,
                                ap=[[0, 128]] + list(row.ap)),
                )
                qg_b.append(t)
            w_sb = {}
            for name, dram in (("q", wqT_d), ("k", wkT_d), ("v", wvT_d)):
                t = consts.tile([128, DC, H], F32R, tag=f"w_{name}")
                nc.sync.dma_start(out=t, in_=dram[:, :].rearrange("p (c h) -> p c h", c=DC))
                w_sb[name] = t
            b_sb = {}
            for name, dram in (("q", bq_d), ("k", bk_d), ("v", bv_d)):
                t = consts.tile([H, 1], F32, tag=f"b_{name}")
                nc.sync.dma_start(out=t, in_=dram[:, :])
                b_sb[name] = t
            KT = big.tile([128, S], F32R, tag="KT")     # K^T [h, k]
            QT = big.tile([128, NQ], F32R, tag="QT")    # Q^T [h, q]

            # ---- interleaved projections + attention ----
            # attention chunk j only needs k-tiles 0..8j+7 (causal), i.e.
            # K/V from s-chunks 0..2j+1 and Q chunk j: project exactly the
            # two new s-chunks per block, then run the chunk's attention.
            # PE therefore has dense work while the remaining x^T streams in.
            with (
                tc.tile_pool(name="vt_sb", bufs=1) as vt_sb_pool,
                tc.tile_pool(name="xp", bufs=16) as xp,
                tc.tile_pool(name="xqp", bufs=8) as xqp,
                tc.tile_pool(name="kps", bufs=1, space="PSUM") as kps,
                tc.tile_pool(name="vps", bufs=1, space="PSUM") as vps,
                tc.tile_pool(name="qps", bufs=1, space="PSUM") as qps,
                tc.tile_pool(name="tps", bufs=1, space="PSUM") as tps,
                tc.tile_pool(name="sp", bufs=2, space="PSUM") as sp,
                tc.tile_pool(name="op", bufs=1, space="PSUM") as op,
                tc.tile_pool(name="dp", bufs=1, space="PSUM") as dp,
                tc.tile_pool(name="mp", bufs=3) as mp,
                tc.tile_pool(name="osb", bufs=2) as osb,
                tc.tile_pool(name="rp", bufs=2) as rp,
            ):
                VT = vt_sb_pool.tile([128, S], F32, tag="VT")
                v_t = [None] * NKT

                def project_sc(sc):
                    kpsum = kps.tile([128, 512], F32)
                    vpsum = vps.tile([128, 512], F32)
                    for dc in range(DC):
                        xt = xp.tile([128, 512], F32R, tag="xt")
                        nc.sync.dma_start(
                            out=xt,
                            in_=xT_d[dc * 128:(dc + 1) * 128, sc * 512:(sc + 1) * 512],
                        )
                        nc.tensor.matmul(kpsum, w_sb["k"][:, dc, :], xt,
                                         start=(dc == 0), stop=(dc == DC - 1))
                        nc.tensor.matmul(vpsum, w_sb["v"][:, dc, :], xt,
                                         start=(dc == 0), stop=(dc == DC - 1))
                    nc.vector.tensor_scalar_add(
                        KT[:, sc * 512:(sc + 1) * 512], kpsum, b_sb["k"])
                    nc.vector.tensor_scalar_add(
                        VT[:, sc * 512:(sc + 1) * 512], vpsum, b_sb["v"])
                    for kt in range(4 * sc, 4 * sc + 4):
                        tpsum = tps.tile([128, 128], F32)
                        nc.tensor.transpose(
                            tpsum, VT[:, kt * 128:(kt + 1) * 128], ident)
                        vt = vtiles.tile([128, H], F32R, tag="v_t")
                        nc.vector.tensor_copy(vt, tpsum)
                        v_t[kt] = vt

                def project_q(jc):
                    qpsum = qps.tile([128, 512], F32)
                    for dc in range(DC):
                        xqt = xqp.tile([128, 512], F32R, tag="xqt")
                        nc.sync.dma_start(
                            out=xqt,
                            in_=xqT_d[dc * 128:(dc + 1) * 128, jc * 512:(jc + 1) * 512],
                        )
                        nc.tensor.matmul(qpsum, w_sb["q"][:, dc, :], xqt,
                                         start=(dc == 0), stop=(dc == DC - 1))
                    nc.vector.tensor_scalar_add(
                        QT[:, jc * 512:(jc + 1) * 512], qpsum, b_sb["q"])

                for j in range(4):
                    project_sc(2 * j)
                    project_q(j)
                    project_sc(2 * j + 1)

                    n_kt = 8 * j + 8
                    outp = op.tile([128, 512], F32)
                    denp = dp.tile([128, 512], F32)
                    pts = [None] * n_kt

                    def score_exp(kt, j=j, pts=pts):
                        spsum = sp.tile([128, 512], F32)
                        nc.tensor.matmul(
                            spsum, KT[:, kt * 128:(kt + 1) * 128],
                            QT[:, j * 512:(j + 1) * 512], start=True, stop=True)
                        pt = ptp.tile([128, 512], F32R, tag="pt")
                        nc.scalar.activation(
                            pt, spsum, mybir.ActivationFunctionType.Exp,
                            bias=mb[:, kt:kt + 1], scale=1.0)
                        if kt >= 8 * j:
                            m_t = mp.tile([128, 512], F32, tag="m")
                            nc.gpsimd.tensor_scalar(
                                m_t, qg_b[j], kg[:, kt:kt + 1], None,
                                mybir.AluOpType.is_ge)
                            pt2 = pt2p.tile([128, 512], F32R, tag="pt2")
                            nc.vector.tensor_mul(pt2, pt, m_t)
                            pts[kt] = pt2
                        else:
                            pts[kt] = pt

                    def pv(kt, j=j, pts=pts, outp=outp, denp=denp, n_kt=n_kt):
                        nc.tensor.matmul(outp, v_t[kt], pts[kt],
                                         start=(kt == 0), stop=(kt == n_kt - 1))
                        nc.tensor.matmul(denp, ones, pts[kt],
                                         start=(kt == 0), stop=(kt == n_kt - 1))

                    # software-pipelined: PE does scores(kt+1) while the
                    # scalar engine exps scores(kt); PV lags one step
                    score_exp(0)
                    for kt in range(1, n_kt):
                        score_exp(kt)
                        pv(kt - 1)
                    pv(n_kt - 1)

                    r_t = rp.tile([128, 512], F32, tag="r")
                    nc.vector.reciprocal(r_t, denp)
                    o_sb = osb.tile([128, 512], F32, tag="o")
                    nc.vector.tensor_mul(o_sb, outp, r_t)
                    nc.sync.dma_start(
                        out=o_d[:, j * 512:(j + 1) * 512], in_=o_sb)
    nc.compile()
    return nc


def check_matmul_waits(nc, limit=1):
    bad = []
    for f in nc.m.functions:
        for bb in f.blocks:
            for i in bb.instructions:
                if i.opcode == "Matmult":
                    w = i.sync_info.on_wait if i.sync_info else []
                    if len(w) > limit:
                        bad.append((i.name, [(x.ant_name, x.wait_value) for x in w]))
    return bad


_NC_CACHE = {}


def _get_program():
    if "nc" not in _NC_CACHE:
        _NC_CACHE["nc"] = _build_program()
    return _NC_CACHE["nc"]


def _make_in_maps(x, mask, Wq, bq, Wk, bk, Wv, bv):
    x = np.asarray(x, np.float32)
    mask = np.asarray(mask)
    scale = 1.0 / np.sqrt(np.float32(H))
    def pack_w(w):
        # [H,D] -> w.T [D,H] -> partition-major [128, DC*H] for a single
        # contiguous-burst DMA into the SBUF weight tile
        wT = np.asarray(w, np.float32).T.reshape(DC, 128, H)
        return np.ascontiguousarray(wT.transpose(1, 0, 2).reshape(128, DC * H))

    wqT = pack_w(np.asarray(Wq, np.float32) * scale)
    wkT = pack_w(Wk)
    wvT = pack_w(Wv)
    bq_c = (np.asarray(bq, np.float32) * scale).reshape(H, 1).copy()
    bk_c = np.asarray(bk, np.float32).reshape(H, 1).copy()
    bv_c = np.asarray(bv, np.float32).reshape(H, 1).copy()

    in_maps = []
    for c in range(8):
        b, p = c // 2, c % 2
        xT = np.ascontiguousarray(x[b].T)                      # [D, S]
        gt = 2 * np.arange(16) + p                             # owned global q-tiles
        cols = (gt[:, None] * 128 + np.arange(128)[None, :]).reshape(-1)
        xqT = np.ascontiguousarray(xT[:, cols])                # [D, NQ]
        mbias = np.where(mask[b] == 0, np.float32(NEG), np.float32(0.0))
        mbias = np.ascontiguousarray(mbias.reshape(NKT, 128).T.astype(np.float32))
        qg = cols.reshape(4, 512).astype(np.float32)
        in_maps.append({
            "xT": xT, "xqT": xqT, "wqT": wqT, "wkT": wkT, "wvT": wvT,
            "bq": bq_c, "bk": bk_c, "bv": bv_c, "maskbias": mbias,
            "qg": np.ascontiguousarray(qg),
        })
    return in_maps


def _install_ntff_hook():
    # the trimmed antenv package lacks axon_hooks; recreate it and wire the
    # ctypes NTFF profiling hook from trn_agent_boot so trace=True works
    import types
    if "antenv.axon_hooks" in sys.modules:
        return
    import antenv
    mod = types.ModuleType("antenv.axon_hooks")
    _hook = [None]
    mod.set_axon_ntff_profile_hook = lambda h: _hook.__setitem__(0, h)
    mod.get_axon_ntff_profile_hook = lambda: _hook[0]
    sys.modules["antenv.axon_hooks"] = mod
    antenv.axon_hooks = mod
    from trn_agent_boot.trn_boot import _ntff_profile_via_ctypes
    mod.set_axon_ntff_profile_hook(
        _ntff_profile_via_ctypes("/opt/axon/libaxon_pjrt.so"))


def run(inputs, trace=False, tmpdir=None):
    if trace:
        try:
            _install_ntff_hook()
        except Exception as e:
            print("ntff hook install failed:", e)
    nc = _get_program()
    in_maps = _make_in_maps(**inputs)
    res = bass_utils.run_bass_kernel_spmd(
        nc, in_maps, core_ids=list(range(8)), trace=trace, tmpdir=tmpdir)
    out = np.empty((B, S, H), np.float32)
    for c in range(8):
        b, p = c // 2, c % 2
        o = res.results[c]["o"]                                # [H, NQ]
        for lt in range(16):
            g = 2 * lt + p
            out[b, g * 128:(g + 1) * 128, :] = o[:, lt * 128:(lt + 1) * 128].T
    return out, res


def kernel(**inputs) -> np.ndarray:
    out, _ = run(inputs, trace=False)
    return out


# revision 18
# speedup vs baseline: 1.9043x; 1.9043x over previous
"""Masked causal self-attention (single head) on 8 Trainium2 NeuronCores.

Problem: x[4,4096,1024], mask[4,4096] (key padding), Wq/Wk/Wv[128,1024],
bq/bk/bv[128] -> out[4,4096,128]:
    q = x@Wq.T+bq; k = x@Wk.T+bk; v = x@Wv.T+bv
    out = softmax(causal_mask(q@k.T/sqrt(128)) + key_padding) @ v

Sharding (SPMD, one program on 8 cores): core c = (batch b=c//2, parity
p=c%2). Each core computes K/V for its full batch (replicated within the
pair) and handles the interleaved query 128-row tiles {2*t+p : t in 0..15}
— interleaving balances the causal (triangular) work between the pair.

Device algorithm per core:
  - All matmuls run in float32r (single-pass fp32, 1 cycle/row at moving
    free-dim >= 256, vs 4 cycles/row for exact fp32) via AP bitcasts.
  - Projections are computed transposed ([head, seq] layout) with the
    d-contraction on partitions: K^T/V^T/Q^T = W.T-chunks @ x^T-chunks,
    accumulated in PSUM over 8 d-chunks. Biases are folded in during the
    PSUM->SBUF eviction (per-partition scalar add). The 1/sqrt(128) score
    scale is folded into Wq on the host.
  - V^T is transposed back to [seq,head] tiles with the PE (stationary
    operand of attn@V).
  - Scores are computed transposed: S^T[k,q] = (K^T-tile).T @ Q^T-chunk.
    exp() runs on the scalar engine straight out of PSUM; its per-partition
    bias argument carries the key-padding mask (-1e30 for masked keys).
    Softmax max-subtraction is skipped: scores are ~N(0,1) by construction
    (matches jax softmax mathematically; no overflow in fp32).
  - The causal mask is data-driven so the program is core-uniform: a 0/1
    tile M = (qg >= kg) (qg per-core query indices from DRAM, kg an iota)
    multiplies exp(S^T) for the ~diagonal k-tiles only (into a fresh tile,
    pt2, so every tile has a single writer engine).
  - attn@V accumulates transposed, a whole 512-query chunk at a time:
    outT[h, qchunk] += V_kt.T @ PT_kt and den[*, qchunk] += ones.T @ PT_kt
    (the ones-matmul gives the softmax denominator replicated across
    partitions, so normalization is a plain elementwise multiply).
    The output leaves the device as [H, NQ]; the host transposes.

Hardware instructions carry a single semaphore-wait slot; Bacc.compile()
legalizes multi-wait instructions (generate_event_semaphores).
"""

import sys

sys.path.insert(0, "/opt/trn_rl_repo")

import numpy as np

import concourse.bass as bass
import concourse.bacc as bacc
import concourse.tile as tile
from concourse import mybir
from concourse.masks import make_identity
from concourse import bass_utils

F32 = mybir.dt.float32
F32R = mybir.dt.float32r
B, S, D, H = 4, 4096, 1024, 128
NQ = S // 2          # queries owned per core (2048)
DC = D // 128        # 8 d-chunks
SCH = S // 512       # 8 seq chunks of 512
NKT = S // 128       # 32 key tiles
NEG = -1.0e30


def _build_program():
    nc = bacc.Bacc("TRN2", target_bir_lowering=False)

    xT_d = nc.dram_tensor("xT", [D, S], F32R, kind="ExternalInput")
    xqT_d = nc.dram_tensor("xqT", [D, NQ], F32R, kind="ExternalInput")
    wqT_d = nc.dram_tensor("wqT", [128, DC * H], F32R, kind="ExternalInput")
    wkT_d = nc.dram_tensor("wkT", [128, DC * H], F32R, kind="ExternalInput")
    wvT_d = nc.dram_tensor("wvT", [128, DC * H], F32R, kind="ExternalInput")
    bq_d = nc.dram_tensor("bq", [H, 1], F32, kind="ExternalInput")
    bk_d = nc.dram_tensor("bk", [H, 1], F32, kind="ExternalInput")
    bv_d = nc.dram_tensor("bv", [H, 1], F32, kind="ExternalInput")
    mb_d = nc.dram_tensor("maskbias", [128, NKT], F32, kind="ExternalInput")
    qg_d = nc.dram_tensor("qg", [4, 512], F32, kind="ExternalInput")
    o_d = nc.dram_tensor("o", [H, NQ], F32, kind="ExternalOutput")

    with tile.TileContext(nc) as tc:
        with (
            tc.tile_pool(name="consts", bufs=1) as consts,
            tc.tile_pool(name="big", bufs=1) as big,
            tc.tile_pool(name="vtiles", bufs=NKT) as vtiles,
            tc.tile_pool(name="ptp", bufs=6) as ptp,
            tc.tile_pool(name="pt2p", bufs=4) as pt2p,
        ):
            # ---- constants ----
            ident = consts.tile([128, 128], F32)
            make_identity(nc, ident)
            ones_f = consts.tile([128, 128], F32, tag="ones_f")
            nc.vector.memset(ones_f, 1.0)
            ones = consts.tile([128, 128], F32R)
            nc.vector.tensor_copy(ones, ones_f)
            kg = consts.tile([128, NKT], F32)
            nc.gpsimd.iota(
                kg, pattern=[[128, NKT]], base=0, channel_multiplier=1,
                allow_small_or_imprecise_dtypes=True,
            )
            mb = consts.tile([128, NKT], F32)
            nc.sync.dma_start(out=mb, in_=mb_d[:, :])
            qg_b = []
            for j in range(4):
                t = consts.tile([128, 512], F32, tag="qg_b")
                row = qg_d[j, :]
                nc.gpsimd.dma_start(
                    out=t,
                    in_=bass.AP(tensor=row.tensor, offset=row.offset,
                                ap=[[0, 128]] + list(row.ap)),
                )
                qg_b.append(t)
            w_sb = {}
            for name, dram in (("q", wqT_d), ("k", wkT_d), ("v", wvT_d)):
                t = consts.tile([128, DC, H], F32R, tag=f"w_{name}")
                nc.sync.dma_start(out=t, in_=dram[:, :].rearrange("p (c h) -> p c h", c=DC))
                w_sb[name] = t
            b_sb = {}
            for name, dram in (("q", bq_d), ("k", bk_d), ("v", bv_d)):
                t = consts.tile([H, 1], F32, tag=f"b_{name}")
                nc.sync.dma_start(out=t, in_=dram[:, :])
                b_sb[name] = t
            KT = big.tile([128, S], F32R, tag="KT")     # K^T [h, k]
            QT = big.tile([128, NQ], F32R, tag="QT")    # Q^T [h, q]

            # ---- interleaved projections + attention ----
            # attention chunk j only needs k-tiles 0..8j+7 (causal), i.e.
            # K/V from s-chunks 0..2j+1 and Q chunk j: project exactly the
            # two new s-chunks per block, then run the chunk's attention.
            # PE therefore has dense work while the remaining x^T streams in.
            with (
                tc.tile_pool(name="vt_sb", bufs=1) as vt_sb_pool,
                tc.tile_pool(name="xp", bufs=16) as xp,
                tc.tile_pool(name="xqp", bufs=8) as xqp,
                tc.tile_pool(name="kps", bufs=1, space="PSUM") as kps,
                tc.tile_pool(name="vps", bufs=1, space="PSUM") as vps,
                tc.tile_pool(name="qps", bufs=1, space="PSUM") as qps,
                tc.tile_pool(name="tps", bufs=1, space="PSUM") as tps,
                tc.tile_pool(name="sp", bufs=2, space="PSUM") as sp,
                tc.tile_pool(name="op", bufs=1, space="PSUM") as op,
                tc.tile_pool(name="dp", bufs=1, space="PSUM") as dp,
                tc.tile_pool(name="mp", bufs=3) as mp,
                tc.tile_pool(name="osb", bufs=2) as osb,
                tc.tile_pool(name="rp", bufs=2) as rp,
            ):
                VT = vt_sb_pool.tile([128, S], F32, tag="VT")
                v_t = [None] * NKT

                def project_sc(sc):
                    kpsum = kps.tile([128, 512], F32)
                    vpsum = vps.tile([128, 512], F32)
                    for dc in range(DC):
                        xt = xp.tile([128, 512], F32R, tag="xt")
                        nc.sync.dma_start(
                            out=xt,
                            in_=xT_d[dc * 128:(dc + 1) * 128, sc * 512:(sc + 1) * 512],
                        )
                        nc.tensor.matmul(kpsum, w_sb["k"][:, dc, :], xt,
                                         start=(dc == 0), stop=(dc == DC - 1))
                        nc.tensor.matmul(vpsum, w_sb["v"][:, dc, :], xt,
                                         start=(dc == 0), stop=(dc == DC - 1))
                    nc.vector.tensor_scalar_add(
                        KT[:, sc * 512:(sc + 1) * 512], kpsum, b_sb["k"])
                    nc.vector.tensor_scalar_add(
                        VT[:, sc * 512:(sc + 1) * 512], vpsum, b_sb["v"])
                    for kt in range(4 * sc, 4 * sc + 4):
                        tpsum = tps.tile([128, 128], F32)
                        nc.tensor.transpose(
                            tpsum, VT[:, kt * 128:(kt + 1) * 128], ident)
                        vt = vtiles.tile([128, H], F32R, tag="v_t")
                        nc.vector.tensor_copy(vt, tpsum)
                        v_t[kt] = vt

                def project_q(jc):
                    qpsum = qps.tile([128, 512], F32)
                    for dc in range(DC):
                        xqt = xqp.tile([128, 512], F32R, tag="xqt")
                        nc.sync.dma_start(
                            out=xqt,
                            in_=xqT_d[dc * 128:(dc + 1) * 128, jc * 512:(jc + 1) * 512],
                        )
                        nc.tensor.matmul(qpsum, w_sb["q"][:, dc, :], xqt,
                                         start=(dc == 0), stop=(dc == DC - 1))
                    nc.vector.tensor_scalar_add(
                        QT[:, jc * 512:(jc + 1) * 512], qpsum, b_sb["q"])

                for j in range(4):
                    project_sc(2 * j)
                    project_q(j)
                    project_sc(2 * j + 1)

                    n_kt = 8 * j + 8
                    outp = op.tile([128, 512], F32)
                    denp = dp.tile([128, 512], F32)
                    pts = [None] * n_kt

                    def score_exp(kt, j=j, pts=pts):
                        spsum = sp.tile([128, 512], F32)
                        nc.tensor.matmul(
                            spsum, KT[:, kt * 128:(kt + 1) * 128],
                            QT[:, j * 512:(j + 1) * 512], start=True, stop=True)
                        pt = ptp.tile([128, 512], F32R, tag="pt")
                        nc.scalar.activation(
                            pt, spsum, mybir.ActivationFunctionType.Exp,
                            bias=mb[:, kt:kt + 1], scale=1.0)
                        if kt >= 8 * j:
                            m_t = mp.tile([128, 512], F32, tag="m")
                            nc.vector.tensor_scalar(
                                m_t, qg_b[j], kg[:, kt:kt + 1], None,
                                mybir.AluOpType.is_ge)
                            pt2 = pt2p.tile([128, 512], F32R, tag="pt2")
                            nc.vector.tensor_mul(pt2, pt, m_t)
                            pts[kt] = pt2
                        else:
                            pts[kt] = pt

                    def pv(kt, j=j, pts=pts, outp=outp, denp=denp, n_kt=n_kt):
                        nc.tensor.matmul(outp, v_t[kt], pts[kt],
                                         start=(kt == 0), stop=(kt == n_kt - 1))
                        nc.tensor.matmul(denp, ones, pts[kt],
                                         start=(kt == 0), stop=(kt == n_kt - 1))

                    # software-pipelined: PE does scores(kt+1) while the
                    # scalar engine exps scores(kt); PV lags one step
                    score_exp(0)
                    for kt in range(1, n_kt):
                        score_exp(kt)
                        pv(kt - 1)
                    pv(n_kt - 1)

                    r_t = rp.tile([128, 512], F32, tag="r")
                    nc.vector.reciprocal(r_t, denp)
                    o_sb = osb.tile([128, 512], F32, tag="o")
                    nc.vector.tensor_mul(o_sb, outp, r_t)
                    nc.sync.dma_start(
                        out=o_d[:, j * 512:(j + 1) * 512], in_=o_sb)
    nc.compile()
    return nc


def check_matmul_waits(nc, limit=1):
    bad = []
    for f in nc.m.functions:
        for bb in f.blocks:
            for i in bb.instructions:
                if i.opcode == "Matmult":
                    w = i.sync_info.on_wait if i.sync_info else []
                    if len(w) > limit:
                        bad.append((i.name, [(x.ant_name, x.wait_value) for x in w]))
    return bad


_NC_CACHE = {}


def _get_program():
    if "nc" not in _NC_CACHE:
        _NC_CACHE["nc"] = _build_program()
    return _NC_CACHE["nc"]


def _make_in_maps(x, mask, Wq, bq, Wk, bk, Wv, bv):
    x = np.asarray(x, np.float32)
    mask = np.asarray(mask)
    scale = 1.0 / np.sqrt(np.float32(H))
    def pack_w(w):
        # [H,D] -> w.T [D,H] -> partition-major [128, DC*H] for a single
        # contiguous-burst DMA into the SBUF weight tile
        wT = np.asarray(w, np.float32).T.reshape(DC, 128, H)
        return np.ascontiguousarray(wT.transpose(1, 0, 2).reshape(128, DC * H))

    wqT = pack_w(np.asarray(Wq, np.float32) * scale)
    wkT = pack_w(Wk)
    wvT = pack_w(Wv)
    bq_c = (np.asarray(bq, np.float32) * scale).reshape(H, 1).copy()
    bk_c = np.asarray(bk, np.float32).reshape(H, 1).copy()
    bv_c = np.asarray(bv, np.float32).reshape(H, 1).copy()

    in_maps = []
    for c in range(8):
        b, p = c // 2, c % 2
        xT = np.ascontiguousarray(x[b].T)                      # [D, S]
        gt = 2 * np.arange(16) + p                             # owned global q-tiles
        cols = (gt[:, None] * 128 + np.arange(128)[None, :]).reshape(-1)
        xqT = np.ascontiguousarray(xT[:, cols])                # [D, NQ]
        mbias = np.where(mask[b] == 0, np.float32(NEG), np.float32(0.0))
        mbias = np.ascontiguousarray(mbias.reshape(NKT, 128).T.astype(np.float32))
        qg = cols.reshape(4, 512).astype(np.float32)
        in_maps.append({
            "xT": xT, "xqT": xqT, "wqT": wqT, "wkT": wkT, "wvT": wvT,
            "bq": bq_c, "bk": bk_c, "bv": bv_c, "maskbias": mbias,
            "qg": np.ascontiguousarray(qg),
        })
    return in_maps


def _install_ntff_hook():
    # the trimmed antenv package lacks axon_hooks; recreate it and wire the
    # ctypes NTFF profiling hook from trn_agent_boot so trace=True works
    import types
    if "antenv.axon_hooks" in sys.modules:
        return
    import antenv
    mod = types.ModuleType("antenv.axon_hooks")
    _hook = [None]
    mod.set_axon_ntff_profile_hook = lambda h: _hook.__setitem__(0, h)
    mod.get_axon_ntff_profile_hook = lambda: _hook[0]
    sys.modules["antenv.axon_hooks"] = mod
    antenv.axon_hooks = mod
    from trn_agent_boot.trn_boot import _ntff_profile_via_ctypes
    mod.set_axon_ntff_profile_hook(
        _ntff_profile_via_ctypes("/opt/axon/libaxon_pjrt.so"))


def run(inputs, trace=False, tmpdir=None):
    if trace:
        try:
            _install_ntff_hook()
        except Exception as e:
            print("ntff hook install failed:", e)
    nc = _get_program()
    in_maps = _make_in_maps(**inputs)
    res = bass_utils.run_bass_kernel_spmd(
        nc, in_maps, core_ids=list(range(8)), trace=trace, tmpdir=tmpdir)
    out = np.empty((B, S, H), np.float32)
    for c in range(8):
        b, p = c // 2, c % 2
        o = res.results[c]["o"]                                # [H, NQ]
        for lt in range(16):
            g = 2 * lt + p
            out[b, g * 128:(g + 1) * 128, :] = o[:, lt * 128:(lt + 1) * 128].T
    return out, res


def kernel(**inputs) -> np.ndarray:
    out, _ = run(inputs, trace=False)
    return out
